# revision 1
# baseline (speedup 1.0000x reference)
"""Causal multi-head attention on 8 TRN2 NeuronCores.

Problem: B=2, T=2048, C=2048, H=16 heads, D=128 head_dim, fp32 reference.

Sharding (hardcoded): tensor-parallel over heads x4 (4 heads per core),
data-parallel over batch x2.  Core i handles batch i//4, head-group i%4
(heads 4*(i%4) .. 4*(i%4)+3).  Each core computes a *partial* output
[T, C] = (softmax(QK^T/sqrt(D)) V)_heads @ wo_shard^T ; the host sums the
4 TP partials per batch (the row-parallel wo all-reduce, done at unshard).

On-chip layout: all matmuls are out = lhsT.T @ rhs with the contraction
dim on SBUF partitions.  The host pre-transposes x and the weights so no
on-chip transposes are ever needed:
  QK^T scores are computed directly as S^T[k, q] (keys on partitions), so
  the softmax denominator l[q] = sum_k exp(S^T) is a ones-vector matmul on
  the PE, and P^T = exp(S^T) feeds the PV matmul (O^T = V.T @ P.T) as-is.
  Causality: k-tiles above the diagonal are skipped; diagonal-band tiles
  are masked multiplicatively after exp.  Scores are ~N(0,1) so exp
  without max-subtraction is numerically safe.
"""

import math

import ml_dtypes
import numpy as np

import concourse.bass as bass
import concourse.tile as tile
from concourse import bacc, mybir
from concourse.bass_utils import run_bass_kernel_spmd

B, T, C = 2, 2048, 2048
H, D = 16, 128
HG = 4              # head-groups (TP degree); heads per core = H // HG = 4
NH = H // HG        # heads per core
NT = T // 512       # 512-wide t/q chunks
SCALE = 1.0 / math.sqrt(D)

BF16 = mybir.dt.bfloat16
F32 = mybir.dt.float32
F32R = mybir.dt.float32r

NP_BF16 = ml_dtypes.bfloat16

# matmul dtypes (bf16 = full PE rate; PSUM accumulation is always f32)
DT_PROJ = BF16      # x, wq/wk/wv
DT_ATT = BF16       # v, p(=exp), masks
DT_S = F32R         # qT, kT: S-matmul operands (f32r = full PE rate, ~13 mantissa bits)
DT_WO = BF16        # oT, woT
DT_R = F32R         # reciprocal + broadcast path (keeps normalizer accurate)

_NP_OF = {BF16: NP_BF16, F32: np.float32, F32R: np.float32}


def _build():
    nc = bacc.Bacc("TRN2", target_bir_lowering=False, debug=False, num_devices=8)

    xt = nc.dram_tensor("xt", [128, 16 * T], DT_PROJ, kind="ExternalInput")
    wqt = nc.dram_tensor("wqt", [128, 8192], DT_PROJ, kind="ExternalInput")
    wkt = nc.dram_tensor("wkt", [128, 8192], DT_PROJ, kind="ExternalInput")
    wvt = nc.dram_tensor("wvt", [128, 8192], DT_PROJ, kind="ExternalInput")
    wot = nc.dram_tensor("wot", [128, 8192], DT_WO, kind="ExternalInput")
    msk = nc.dram_tensor("msk", [128, 2048], DT_ATT, kind="ExternalInput")
    out = nc.dram_tensor("out", [T, C], F32, kind="ExternalOutput")

    with tile.TileContext(nc) as tc:
        with (
            tc.tile_pool(name="big", bufs=1) as big,
            tc.tile_pool(name="xs", bufs=2) as xs,
            tc.tile_pool(name="work", bufs=2) as work,
            tc.tile_pool(name="ps", bufs=2, space="PSUM") as psum,
        ):
            wq_sb = big.tile([128, 8192], DT_PROJ)
            wk_sb = big.tile([128, 8192], DT_PROJ)
            wv_sb = big.tile([128, 8192], DT_PROJ)
            wo_sb = big.tile([128, 8192], DT_WO)
            msk_sb = big.tile([128, 2048], DT_ATT)
            ones_k = big.tile([128, 1], DT_ATT)
            nc.gpsimd.memset(ones_k[:], 1.0)

            # Interleave wq/x0 quarter-DMAs so the first Q matmuls (which only
            # need the first c-tiles) can start as early as possible.
            x0_sb = xs.tile([128, 8192], DT_PROJ, tag="x")
            for q4 in range(4):
                sl = slice(2048 * q4, 2048 * (q4 + 1))
                nc.sync.dma_start(wq_sb[:, sl], wqt[:, sl])
                nc.sync.dma_start(x0_sb[:, sl], xt[:, sl])
            nc.sync.dma_start(wk_sb[:], wkt[:])
            nc.sync.dma_start(wv_sb[:], wvt[:])

            kT_sb = big.tile([128, NH * T], DT_S)   # per head: [d=128, t]
            v_sb = big.tile([128, 16 * 512], DT_ATT)  # [t=128, (t_tile, 4h*128)]
            oT_sb = big.tile([128, NH * T], DT_WO)    # per head: [d=128, t]

            for tci in range(NT):
                if tci == 0:
                    x_sb = x0_sb
                else:
                    x_sb = xs.tile([128, 8192], DT_PROJ, tag="x")
                    nc.sync.dma_start(x_sb[:], xt[:, 8192 * tci:8192 * (tci + 1)])

                # ---- projections for this 512-wide t-chunk ----
                qT = work.tile([128, NH * 512], DT_S, tag="qT")
                for h in range(NH):
                    ps = psum.tile([128, 512], F32, tag="acc", bufs=3)
                    for ci in range(16):
                        nc.tensor.matmul(
                            ps[:],
                            lhsT=wq_sb[:, 512 * ci + 128 * h:512 * ci + 128 * (h + 1)],
                            rhs=x_sb[:, 512 * ci:512 * (ci + 1)],
                            start=(ci == 0), stop=(ci == 15),
                        )
                    nc.scalar.copy(qT[:, 512 * h:512 * (h + 1)], ps[:])
                for h in range(NH):
                    ps = psum.tile([128, 512], F32, tag="acc", bufs=3)
                    for ci in range(16):
                        nc.tensor.matmul(
                            ps[:],
                            lhsT=wk_sb[:, 512 * ci + 128 * h:512 * ci + 128 * (h + 1)],
                            rhs=x_sb[:, 512 * ci:512 * (ci + 1)],
                            start=(ci == 0), stop=(ci == 15),
                        )
                    nc.scalar.copy(
                        kT_sb[:, T * h + 512 * tci:T * h + 512 * (tci + 1)], ps[:]
                    )
                for ts in range(4):
                    ps = psum.tile([128, 512], F32, tag="acc", bufs=3)
                    for ci in range(16):
                        nc.tensor.matmul(
                            ps[:],
                            lhsT=x_sb[:, 512 * ci + 128 * ts:512 * ci + 128 * (ts + 1)],
                            rhs=wv_sb[:, 512 * ci:512 * (ci + 1)],
                            start=(ci == 0), stop=(ci == 15),
                        )
                    tt = 4 * tci + ts
                    nc.scalar.copy(v_sb[:, 512 * tt:512 * (tt + 1)], ps[:])

                if tci == 0:
                    # deferred so the first x chunk + weights win the DMA queue
                    nc.sync.dma_start(msk_sb[:], msk[:])
                    nc.sync.dma_start(wo_sb[:], wot[:])

                # ---- causal attention for q-chunk tci, all 4 heads ----
                # One flat software-pipelined stream over (head, k-tile): the
                # S matmul + exp run one step ahead of the dependent l/PV
                # matmuls (so the PE queue never head-blocks on the exp), the
                # pipeline carries across head boundaries, and the previous
                # chunk's WO matmul groups are injected into the stream to
                # fill the exp-gated PE bubbles.
                nk = 4 * tci + 4
                o_ps = {}
                l_ps = {}

                prev_p = {}
                pair1_of = {}
                lpend = []

                def _s_exp(h, kt):
                    s_ps = psum.tile([128, 512], F32, tag="s", bufs=2)
                    nc.tensor.matmul(
                        s_ps[:],
                        lhsT=kT_sb[:, T * h + 128 * kt:T * h + 128 * (kt + 1)],
                        rhs=qT[:, 512 * h:512 * (h + 1)],
                        start=True, stop=True,
                    )
                    p = work.tile([128, 512], DT_ATT, tag="p", bufs=4)
                    nc.scalar.activation(
                        p[:], s_ps[:], mybir.ActivationFunctionType.Exp,
                        scale=SCALE,
                    )
                    if kt >= 4 * tci:
                        m = kt - 4 * tci
                        nc.vector.tensor_mul(
                            p[:], p[:], msk_sb[:, 512 * m:512 * (m + 1)]
                        )
                    # pair-sum consecutive P tiles on DVE so the softmax
                    # denominator needs only one ones-matmul per pair (nk is
                    # always even); the fp32 PSUM accumulation is unaffected.
                    if kt % 2 == 0:
                        prev_p[h] = p
                    else:
                        pp = work.tile([128, 512], DT_ATT, tag="pp", bufs=3)
                        nc.vector.tensor_add(pp[:], prev_p[h][:], p[:])
                        if kt % 4 == 1:
                            pair1_of[h] = pp
                        else:
                            qq = work.tile([128, 512], DT_ATT, tag="qq", bufs=3)
                            nc.vector.tensor_add(qq[:], pair1_of.pop(h)[:], pp[:])
                            lpend.append((h, kt, qq))
                    return p

                def _l_mm(lh, lkt, qq):
                    nc.tensor.matmul(
                        l_ps[lh][:], lhsT=ones_k[:], rhs=qq[:],
                        start=(lkt == 3), stop=(lkt == nk - 1),
                    )

                def _l_pv(h, kt, p):
                    # emit a deferred denominator matmul only once a newer
                    # quad exists, giving its DVE add chain ~4 iterations
                    if len(lpend) > 1:
                        _l_mm(*lpend.pop(0))
                    nc.tensor.matmul(
                        o_ps[h][:],
                        lhsT=v_sb[:, 512 * kt + 128 * h:512 * kt + 128 * (h + 1)],
                        rhs=p[:],
                        start=(kt == 0), stop=(kt == nk - 1),
                    )

                def _epilogue(h):
                    while lpend and lpend[0][0] == h:
                        _l_mm(*lpend.pop(0))
                    r_sb = work.tile([1, 512], F32, tag="r")
                    nc.vector.reciprocal_approx_fast(r_sb[:], l_ps[h][:])
                    rb_sb = work.tile([128, 512], F32, tag="rb")
                    nc.gpsimd.partition_broadcast(rb_sb[:], r_sb[:])
                    nc.vector.tensor_mul(
                        oT_sb[:, T * h + 512 * tci:T * h + 512 * (tci + 1)],
                        o_ps[h][:], rb_sb[:],
                    )

                def _wo_group(wo_tci, ts, cc):
                    t0 = 512 * wo_tci + 128 * ts
                    ps = psum.tile([128, 512], F32, tag="acc", bufs=3)
                    for h in range(NH):
                        nc.tensor.matmul(
                            ps[:],
                            lhsT=oT_sb[:, T * h + t0:T * h + t0 + 128],
                            rhs=wo_sb[:, 2048 * h + 512 * cc:2048 * h + 512 * (cc + 1)],
                            start=(h == 0), stop=(h == NH - 1),
                        )
                    ob = work.tile([128, 512], F32, tag="ob", bufs=3)
                    nc.vector.tensor_copy(ob[:], ps[:])
                    nc.sync.dma_start(
                        out[t0:t0 + 128, 512 * cc:512 * (cc + 1)], ob[:]
                    )

                pending_wo = (
                    [(tci - 1, ts, cc) for ts in range(4) for cc in range(4)]
                    if tci > 0 else []
                )
                pend = []
                for idx, (h, kt) in enumerate(
                    (h, kt) for h in range(NH) for kt in range(nk)
                ):
                    if kt == 0:
                        o_ps[h] = psum.tile([128, 512], F32, tag="o", name="o_ps")
                        l_ps[h] = psum.tile([1, 512], F32, tag="l", name="l_ps", bufs=1)
                    pend.append((h, kt, _s_exp(h, kt)))
                    if len(pend) > 2:
                        ch, ckt, cp = pend.pop(0)
                        _l_pv(ch, ckt, cp)
                        if ckt == nk - 1:
                            _epilogue(ch)
                    if pending_wo and idx % (tci + 1) == tci:
                        _wo_group(*pending_wo.pop(0))
                for ch, ckt, cp in pend:
                    _l_pv(ch, ckt, cp)
                    if ckt == nk - 1:
                        _epilogue(ch)
                for g in pending_wo:
                    _wo_group(*g)

            # final chunk's output projection (tail)
            for ts in range(4):
                for cc in range(4):
                    _wo_group(NT - 1, ts, cc)
    nc.compile()
    return nc


_NC = None


def _get_nc():
    global _NC
    if _NC is None:
        _NC = _build()
    return _NC


def _pack_w(w, hg):
    # wq/wk/wv shard for head-group hg, pre-transposed + tiled:
    # out[p, 512*ci + d] = w[512*hg + d, 128*ci + p]
    wt = np.ascontiguousarray(w[512 * hg:512 * (hg + 1), :].T)  # [C, 512]
    return np.ascontiguousarray(
        wt.reshape(16, 128, 512).transpose(1, 0, 2).reshape(128, 8192)
    )


def _pack_wo(wo, hg):
    # wo columns for head-group hg, transposed + tiled by head:
    # out[p, 2048*h + c] = wo[c, 512*hg + 128*h + p]
    wt = np.ascontiguousarray(wo[:, 512 * hg:512 * (hg + 1)].T)  # [512, C]
    return np.ascontiguousarray(
        wt.reshape(4, 128, 2048).transpose(1, 0, 2).reshape(128, 8192)
    )


def _pack_x(xb):
    # x[b] transposed + tiled: out[p, 8192*tc + 512*ci + tt] = x[512*tc+tt, 128*ci+p]
    xT = np.ascontiguousarray(xb.T)  # [C, T]
    return np.ascontiguousarray(
        xT.reshape(16, 128, 4, 512).transpose(1, 2, 0, 3).reshape(128, 16 * T)
    )


def _diag_masks():
    kk = np.arange(128)[:, None]
    qq = np.arange(512)[None, :]
    blocks = [(128 * m + kk <= qq).astype(np.float32) for m in range(4)]
    return np.concatenate(blocks, axis=1)  # [128, 2048]


def _in_maps(x, wq, wk, wv, wo):
    np_proj = _NP_OF[DT_PROJ]
    np_att = _NP_OF[DT_ATT]
    np_wo = _NP_OF[DT_WO]
    msk = _diag_masks().astype(np_att)
    xts = [_pack_x(x[b]).astype(np_proj) for b in range(B)]
    wqts = [_pack_w(wq, g).astype(np_proj) for g in range(HG)]
    wkts = [_pack_w(wk, g).astype(np_proj) for g in range(HG)]
    wvts = [_pack_w(wv, g).astype(np_proj) for g in range(HG)]
    wots = [_pack_wo(wo, g).astype(np_wo) for g in range(HG)]
    maps = []
    for i in range(8):
        b, g = divmod(i, HG)
        maps.append({
            "xt": xts[b], "wqt": wqts[g], "wkt": wkts[g], "wvt": wvts[g],
            "wot": wots[g], "msk": msk,
        })
    return maps


def _run(x, wq, wk, wv, wo, trace=False):
    nc = _get_nc()
    maps = _in_maps(x, wq, wk, wv, wo)
    res = run_bass_kernel_spmd(nc, maps, core_ids=list(range(8)), trace=trace)
    full = np.empty((B, T, C), dtype=np.float32)
    for b in range(B):
        acc = res.results[HG * b]["out"].astype(np.float32)
        for g in range(1, HG):
            acc = acc + res.results[HG * b + g]["out"]
        full[b] = acc
    return full, res


def kernel(x, mask=None, wq=None, wk=None, wv=None, wo=None, **_ignored):
    x = np.asarray(x, dtype=np.float32)
    wq = np.asarray(wq, dtype=np.float32)
    wk = np.asarray(wk, dtype=np.float32)
    wv = np.asarray(wv, dtype=np.float32)
    wo = np.asarray(wo, dtype=np.float32)
    full, _ = _run(x, wq, wk, wv, wo, trace=False)
    return full

